# revision 4
# baseline (speedup 1.0000x reference)
"""Trainium2 Bass kernel for ActorCriticSNNPolicy (data-parallel over 8 cores).

Architecture notes
------------------
The reference is a 40-step constant-current LIF encoder feeding a 40-step
recurrent LIF layer with leaky-integrator actor/critic readouts.

Key algorithmic facts exploited here:
1. The encoder resets its membrane to exactly 0.0 on spike, so each feature's
   spike train is exactly periodic with period P(c).  P(c) is a threshold
   function of the input current c: P <= d  iff  c > th_d, where th_d are
   universal fp32 constants of the recurrence (bit-exact bisected offline
   against the fp32 iteration).  So encoder spikes z_e[t] = sum_{d | t+1}
   (S_d - S_{d-1}) with S_d = (c > th_d) -- no per-step simulation at all.
2. All matmul right-hand sides are exact {0,1} spikes, so matmuls run as
   PAIRED bf16 matmuls with hi/lo-split fp32 weights (error ~4e-6 relative,
   2x faster than native 4-pass fp32 matmul).
3. The W_in contraction is applied to the S-masks once (batched, N=5120),
   then per-step encoder drive enters PSUM via fp32 identity matmuls of the
   difference tensors Ghat_d, summed over divisors of t+1.
4. Actor/critic readouts are linear in the recurrent spikes, so they are
   evaluated after the loop with one batched matmul plus two
   tensor_tensor_scan recurrences (chain-reset trick: data0 decay pattern
   with 0.0 at each batch row's chain start).
"""

import os
import sys

sys.path.insert(0, "/opt/trn_rl_repo")

import numpy as np

# ---- constants (universal for this module; not input data) ----
TH_BITS = [0x41200000, 0x40A86BCA, 0x406C2991, 0x403A19C2, 0x401C48CB,
           0x400896C8, 0x3FF559AC, 0x3FE0BEE2, 0x3FD0F3D5, 0x3FC48605,
           0x3FBA8995, 0x3FB26134, 0x3FAB9FEC, 0x3FA5F7DC, 0x3FA12FDD,
           0x3F9D1D0A, 0x3F999E90, 0x3F969AE1, 0x3F93FDD1, 0x3F91B733,
           0x3F8FB9F6, 0x3F8DFB62, 0x3F8C72A3, 0x3F8B185A, 0x3F89E659,
           0x3F88D769, 0x3F87E718, 0x3F87119D, 0x3F8653B4, 0x3F85AA91,
           0x3F8513C1, 0x3F848D28, 0x3F8414ED, 0x3F83A971, 0x3F83494A,
           0x3F82F339, 0x3F82A624, 0x3F826111, 0x3F822324, 0x3F81EB9C]
TH = np.array(TH_BITS, dtype=np.uint32).view(np.float32)

T_ENC = 40
B_LOCAL = 128          # batch rows per core
H = 128                # hidden
A = 19                 # actor(18) + critic(1) rows
N_CORES = 8

DIVS = [[d for d in range(1, 41) if (t + 1) % d == 0] for t in range(T_ENC)]

_CACHE = {}


def _build():
    import concourse.bass as bass
    import concourse.mybir as mybir
    import concourse.tile as tile
    from concourse import bacc
    from concourse.masks import make_identity

    f32 = mybir.dt.float32
    bf16 = mybir.dt.bfloat16
    Alu = mybir.AluOpType
    Act = mybir.ActivationFunctionType

    nc = bacc.Bacc("TRN2", target_bir_lowering=False, debug=False,
                   num_devices=N_CORES)

    xt_d = nc.dram_tensor("xt", [256, B_LOCAL], f32, kind="ExternalInput")
    w_in_d = nc.dram_tensor("w_in_t", [512, H], f32, kind="ExternalInput")
    w_rec_d = nc.dram_tensor("w_rec_t", [H, H], f32, kind="ExternalInput")
    w_ac_d = nc.dram_tensor("w_ac_t", [H, A], f32, kind="ExternalInput")
    out_d = nc.dram_tensor("out", [B_LOCAL, A], f32, kind="ExternalOutput")

    with tile.TileContext(nc) as tc:
        with (
            tc.tile_pool(name="persist", bufs=1) as pp,
            tc.tile_pool(name="smask", bufs=2) as sp,
            tc.tile_pool(name="vstate", bufs=3) as vp,
            tc.tile_pool(name="istate", bufs=3) as ip,
            tc.tile_pool(name="vdec", bufs=3) as vdp,
            tc.tile_pool(name="nzp", bufs=3) as nzp,
            tc.tile_pool(name="psum_g", bufs=2, space="PSUM") as pgp,
            tc.tile_pool(name="psum_i", bufs=3, space="PSUM") as pip,
            tc.tile_pool(name="psum_ac", bufs=2, space="PSUM") as pacp,
            tc.tile_pool(name="psum_t", bufs=1, space="PSUM") as ptp,
        ):
            # ---------------- prep: DMAs ----------------
            xt = pp.tile([128, 2, 128], f32, tag="xt")
            nc.sync.dma_start(xt[:, 0, :], xt_d[0:128, :])
            nc.sync.dma_start(xt[:, 1, :], xt_d[128:256, :])

            w_in = pp.tile([128, 4, H], f32, tag="w_in")
            for j in range(4):
                nc.sync.dma_start(w_in[:, j, :], w_in_d[128 * j:128 * (j + 1), :])
            w_rec = pp.tile([128, H], f32, tag="w_rec")
            nc.sync.dma_start(w_rec[:], w_rec_d[:, :])
            w_ac = pp.tile([128, A], f32, tag="w_ac")
            nc.sync.dma_start(w_ac[:], w_ac_d[:, :])

            # ---------------- prep: scale by 0.1 and hi/lo split ----------------
            def hilo(dst_hi, dst_lo, src, scratch):
                # src fp32 [128, n]; dst_* bf16 [128, n]
                nc.vector.tensor_scalar(scratch[:], src[:], 0.1, None, op0=Alu.mult)
                nc.vector.tensor_copy(dst_hi[:], scratch[:])
                nc.vector.tensor_tensor(scratch[:], scratch[:], dst_hi[:],
                                        op=Alu.subtract)
                nc.vector.tensor_copy(dst_lo[:], scratch[:])

            w_in_hi = pp.tile([128, 4, H], bf16, tag="w_in_hi")
            w_in_lo = pp.tile([128, 4, H], bf16, tag="w_in_lo")
            scr_in = pp.tile([128, 4, H], f32, tag="scr_in")
            hilo(w_in_hi, w_in_lo, w_in, scr_in)

            w_rec_hi = pp.tile([128, H], bf16, tag="w_rec_hi")
            w_rec_lo = pp.tile([128, H], bf16, tag="w_rec_lo")
            scr_rec = pp.tile([128, H], f32, tag="scr_rec")
            hilo(w_rec_hi, w_rec_lo, w_rec, scr_rec)

            w_ac_hi = pp.tile([128, A], bf16, tag="w_ac_hi")
            w_ac_lo = pp.tile([128, A], bf16, tag="w_ac_lo")
            scr_ac = pp.tile([128, A], f32, tag="scr_ac")
            hilo(w_ac_hi, w_ac_lo, w_ac, scr_ac)

            i128 = pp.tile([128, 128], f32, tag="i128")
            make_identity(nc, i128[:])
            i19 = pp.tile([A, A], f32, tag="i19")
            make_identity(nc, i19[:])

            # ---------------- currents c = relu(+-50 x), feature-major ----------
            c4 = pp.tile([128, 4, 128], f32, tag="c4")
            nc.scalar.activation(c4[:, 0, :], xt[:, 0, :], Act.Relu, scale=50.0)
            nc.scalar.activation(c4[:, 1, :], xt[:, 1, :], Act.Relu, scale=50.0)
            nc.scalar.activation(c4[:, 2, :], xt[:, 0, :], Act.Relu, scale=-50.0)
            nc.scalar.activation(c4[:, 3, :], xt[:, 1, :], Act.Relu, scale=-50.0)

            # ---------------- S masks + batched G matmuls (chunked by 4 d) -----
            g_raw = pp.tile([128, T_ENC, B_LOCAL], f32, tag="g_raw")
            for c in range(10):
                s_chunk = sp.tile([128, 4, 4, 128], bf16, tag="s_chunk")
                for k in range(4):
                    d = 4 * c + k + 1
                    eng = nc.vector if (k % 2 == 0) else nc.gpsimd
                    eng.tensor_scalar(s_chunk[:, k, :, :], c4[:, :, :],
                                      float(TH[d - 1]), None, op0=Alu.is_gt)
                pg = pgp.tile([128, 4, 128], f32, tag="pg")
                n_mm = 8
                mm = 0
                for j in range(4):
                    for w in (w_in_hi, w_in_lo):
                        nc.tensor.matmul(pg[:], w[:, j, :], s_chunk[:, :, j, :],
                                         start=(mm == 0), stop=(mm == n_mm - 1))
                        mm += 1
                nc.scalar.copy(g_raw[:, 4 * c:4 * c + 4, :], pg[:])

            # Ghat_d = G_d - G_{d-1}  (slot 0 is G_1 itself)
            ghat = pp.tile([128, T_ENC, B_LOCAL], f32, tag="ghat")
            nc.scalar.copy(ghat[:, 0, :], g_raw[:, 0, :])
            nc.vector.tensor_tensor(ghat[:, 1:20, :], g_raw[:, 1:20, :],
                                    g_raw[:, 0:19, :], op=Alu.subtract)
            nc.gpsimd.tensor_tensor(ghat[:, 20:40, :], g_raw[:, 20:40, :],
                                    g_raw[:, 19:39, :], op=Alu.subtract)

            # ---------------- recurrent loop ----------------
            z_all = pp.tile([128, T_ENC, B_LOCAL], bf16, tag="z_all")
            v_cur = vp.tile([128, B_LOCAL], f32, tag="v0")
            i_cur = ip.tile([128, B_LOCAL], f32, tag="i0")
            nc.vector.memset(v_cur[:], 0.0)
            nc.vector.memset(i_cur[:], 0.0)

            for t in range(T_ENC):
                pi = pip.tile([128, B_LOCAL], f32, tag="pi")
                divs = DIVS[t]
                n_mm = len(divs) + (2 if t > 0 else 0)
                mm = 0
                for d in divs:
                    nc.tensor.matmul(pi[:], i128[:], ghat[:, d - 1, :],
                                     start=(mm == 0), stop=(mm == n_mm - 1))
                    mm += 1
                if t > 0:
                    for w in (w_rec_hi, w_rec_lo):
                        nc.tensor.matmul(pi[:], w[:], z_all[:, t - 1, :],
                                         start=False, stop=(mm == n_mm - 1))
                        mm += 1

                # v_dec = 0.9*v + i~   (uses previous-step state only)
                vd = vdp.tile([128, B_LOCAL], f32, tag="vd")
                nc.vector.scalar_tensor_tensor(vd[:], v_cur[:], 0.9, i_cur[:],
                                               op0=Alu.mult, op1=Alu.add)
                # z = (v_dec > 1)  -> bf16 spike slot
                nc.vector.tensor_scalar(z_all[:, t, :], vd[:], 1.0, None,
                                        op0=Alu.is_gt)
                # nz = (v_dec <= 1)
                nz = nzp.tile([128, B_LOCAL], bf16, tag="nz")
                nc.gpsimd.tensor_scalar(nz[:], vd[:], 1.0, None, op0=Alu.is_le)
                # i~ = 0.8*i~ + psum
                i_new = ip.tile([128, B_LOCAL], f32, tag="i0")
                nc.vector.scalar_tensor_tensor(i_new[:], i_cur[:], 0.8, pi[:],
                                               op0=Alu.mult, op1=Alu.add)
                # v = v_dec * nz
                v_new = vp.tile([128, B_LOCAL], f32, tag="v0")
                nc.vector.tensor_tensor(v_new[:], vd[:], nz[:], op=Alu.mult)
                v_cur, i_cur = v_new, i_new

            # ---------------- actor/critic: batched matmul, (b, t) order -------
            # P_ac_pad[a, b, 0] = 0 ; P_ac_pad[a, b, 1+t] = (0.1 W_ac @ z[t])[a, b]
            pac_pad = pp.tile([A, B_LOCAL, T_ENC + 1], f32, tag="pac_pad")
            nc.vector.memset(pac_pad[:, :, 0:1], 0.0)
            z_bt = z_all[:].rearrange("h t b -> h b t")
            bchunks = [(0, 12)] * 10 + [(120, 8)]
            bchunks = [(12 * i, 12) for i in range(10)] + [(120, 8)]
            for (b0, bn) in bchunks:
                pac = pacp.tile([A, 12, T_ENC], f32, tag="pac")
                nc.tensor.matmul(pac[:, 0:bn, :], w_ac_hi[:], z_bt[:, b0:b0 + bn, :],
                                 start=True, stop=False)
                nc.tensor.matmul(pac[:, 0:bn, :], w_ac_lo[:], z_bt[:, b0:b0 + bn, :],
                                 start=False, stop=True)
                nc.scalar.copy(pac_pad[:, b0:b0 + bn, 1:T_ENC + 1], pac[:, 0:bn, :])

            # ---------------- scans for ia / va ----------------
            SL = B_LOCAL * (T_ENC + 1)  # 5248 flat slots per partition
            dec08 = pp.tile([A, B_LOCAL, T_ENC + 1], f32, tag="dec08")
            nc.gpsimd.memset(dec08[:], 0.8)
            nc.gpsimd.memset(dec08[:, :, 0:1], 0.0)
            dec09 = pp.tile([A, B_LOCAL, T_ENC + 1], f32, tag="dec09")
            nc.gpsimd.memset(dec09[:], 0.9)
            nc.gpsimd.memset(dec09[:, :, 0:2], 0.0)

            ia_arr = pp.tile([A, B_LOCAL, T_ENC + 1], f32, tag="ia_arr")
            va_arr = pp.tile([A, B_LOCAL, T_ENC + 1], f32, tag="va_arr")
            ia_f = ia_arr[:].rearrange("a b t -> a (b t)")
            va_f = va_arr[:].rearrange("a b t -> a (b t)")
            pac_f = pac_pad[:].rearrange("a b t -> a (b t)")
            d08_f = dec08[:].rearrange("a b t -> a (b t)")
            d09_f = dec09[:].rearrange("a b t -> a (b t)")
            half = 64 * (T_ENC + 1)  # 2624

            # ia[t] = 0.8 ia[t-1] + p[t]   (slot k=1+t; k=0 resets chain)
            nc.vector.tensor_tensor_scan(
                ia_f[:, 0:half], d08_f[:, 0:half], pac_f[:, 0:half],
                initial=0.0, op0=Alu.mult, op1=Alu.add)
            nc.vector.tensor_tensor_scan(
                ia_f[:, half:SL], d08_f[:, half:SL], pac_f[:, half:SL],
                initial=0.0, op0=Alu.mult, op1=Alu.add)
            # va[t] = 0.9 va[t-1] + ia[t-1]  (out slot k reads ia slot k-1)
            nc.vector.tensor_tensor_scan(
                va_f[:, 1:half], d09_f[:, 1:half], ia_f[:, 0:half - 1],
                initial=0.0, op0=Alu.mult, op1=Alu.add)
            nc.vector.tensor_tensor_scan(
                va_f[:, half:SL], d09_f[:, half:SL], ia_f[:, half - 1:SL - 1],
                initial=0.0, op0=Alu.mult, op1=Alu.add)

            # m = max_t va  -> [A, B]
            m_ab = pp.tile([A, B_LOCAL], f32, tag="m_ab")
            nc.vector.tensor_reduce(m_ab[:], va_arr[:, :, 1:T_ENC + 1],
                                    axis=mybir.AxisListType.X, op=Alu.max)

            # ---------------- transpose + softmax + output ----------------
            pt = ptp.tile([B_LOCAL, A], f32, tag="pt")
            nc.tensor.transpose(pt[:], m_ab[:], i19[:])
            mt = pp.tile([B_LOCAL, A], f32, tag="mt")
            nc.scalar.copy(mt[:], pt[:])

            mx = pp.tile([B_LOCAL, 1], f32, tag="mx")
            nc.vector.tensor_reduce(mx[:], mt[:, 0:18], axis=mybir.AxisListType.X,
                                    op=Alu.max)
            nmx = pp.tile([B_LOCAL, 1], f32, tag="nmx")
            nc.vector.tensor_scalar(nmx[:], mx[:], -1.0, None, op0=Alu.mult)
            et = pp.tile([B_LOCAL, 18], f32, tag="et")
            nc.scalar.activation(et[:], mt[:, 0:18], Act.Exp, bias=nmx[:])
            s_sum = pp.tile([B_LOCAL, 1], f32, tag="s_sum")
            nc.vector.tensor_reduce(s_sum[:], et[:], axis=mybir.AxisListType.X,
                                    op=Alu.add)
            r_sum = pp.tile([B_LOCAL, 1], f32, tag="r_sum")
            nc.vector.reciprocal(r_sum[:], s_sum[:])

            out_sb = pp.tile([B_LOCAL, A], f32, tag="out_sb")
            nc.vector.tensor_scalar(out_sb[:, 0:18], et[:], r_sum[:], None,
                                    op0=Alu.mult)
            nc.scalar.copy(out_sb[:, 18:19], mt[:, 18:19])
            nc.sync.dma_start(out_d[:, :], out_sb[:])

    nc.compile()
    return nc


def _get_nc():
    if "nc" not in _CACHE:
        _CACHE["nc"] = _build()
    return _CACHE["nc"]


def kernel(x, w_in, w_rec, w_actor, w_critic):
    from concourse.bass_utils import run_bass_kernel_spmd

    x = np.asarray(x, np.float32)
    w_in_t = np.ascontiguousarray(np.asarray(w_in, np.float32).T)       # [512, 128]
    w_rec_t = np.ascontiguousarray(np.asarray(w_rec, np.float32).T)     # [128, 128]
    w_ac = np.concatenate([np.asarray(w_actor, np.float32),
                           np.asarray(w_critic, np.float32)], axis=0)   # [19, 128]
    w_ac_t = np.ascontiguousarray(w_ac.T)                               # [128, 19]

    in_maps = []
    for c in range(N_CORES):
        xs = x[c * B_LOCAL:(c + 1) * B_LOCAL]                           # [128, 256]
        in_maps.append({
            "xt": np.ascontiguousarray(xs.T),
            "w_in_t": w_in_t,
            "w_rec_t": w_rec_t,
            "w_ac_t": w_ac_t,
        })

    nc = _get_nc()
    trace = bool(int(os.environ.get("SNN_KERNEL_TRACE", "0")))
    res = run_bass_kernel_spmd(nc, in_maps, core_ids=list(range(N_CORES)),
                               trace=trace)
    _CACHE["last_result"] = res

    probs = np.empty((1024, 18), np.float32)
    vals = np.empty((1024, 1), np.float32)
    for c in range(N_CORES):
        out = res.results[c]["out"]
        probs[c * B_LOCAL:(c + 1) * B_LOCAL] = out[:, 0:18]
        vals[c * B_LOCAL:(c + 1) * B_LOCAL] = out[:, 18:19]
    return probs, vals
